# revision 5
# baseline (speedup 1.0000x reference)
"""Trainium2 Bass kernel for nn_DistMultMod (DistMult scoring with disease-
neighbor smoothing), distributed over 8 NeuronCores.

Math (see reference): for triplets (h, r, t), relations 2..4 trigger a
smoothing update new_row = c*dv + (1-c)*old on node_emb[h] (last write wins),
then scores = sum_d head*rel*tail read through the updated table.

Decomposition: only the LAST writer triplet of each node survives the
scatter, so per triplet b both lookups reduce to
    vec = a_s * dv_s + g_s * old_row
where s indexes a small per-core table of "comp slots" (masked last-writer
nodes referenced by this core's triplets, ~200 of them), a_s = c, g_s = 1-c
(dummy slot: a=0, g=1).  dv_s = sum_k w[l_s,k] * node_emb[neigh[l_s,k]].

Sharding: data-parallel over the 8192-triplet batch (1024/core); node_emb and
rel_emb replicated; each core computes dv only for its own comp slots.  All
index routing is computed on the host; the device does every float gather /
multiply / reduce.

Device program per core:
  comp phase: 192 indirect row gathers [128,1]->[128,128] from node_emb
    (gather column q covers neighbor lists of slot pair 2q,2q+1), each
    reduced by a PE matmul with a [128,2] weight column into psum rows
    2q..2q+1; psum -> scratch DRAM [384, 132] rows (dv | a | g | pad).
  score phase: indirect gathers of scratch rows (per-triplet head/tail slot),
    old head/tail rows from node_emb and rel rows, then DVE blend+product+
    reduce -> [128, 8] scores per core.
"""
import os
import numpy as np

B = 8192
NCORES = 8
BC = B // NCORES        # 1024 triplets per core
D = 128
K = 64
N = 500000
RELS = 16
ND = 20000
M = 384                 # comp slots per core (3 x 128); last slot is dummy
NT = M // 128           # comp slot tiles
NQ = M // 2             # gather columns (slot pairs) = 192
NJ = BC // 128          # score tiles = 8
SC_W = 132              # scratch row: 128 dv + a + g + 2 pad
LAM = 0.7

_CACHE = {}


# ----------------------------------------------------------------------------
# host-side routing (integer index manipulation only)
# ----------------------------------------------------------------------------
def _prep_cores(head, rel, tailv, local_idx_map, sim_neighbors, sim_weights,
                degree_table):
    mask = (rel >= 2) & (rel <= 4)

    last_of = {}
    for b in range(B):
        last_of[int(head[b])] = b

    cores = []
    for c in range(NCORES):
        lo = c * BC
        slot_of = {}
        writers = []

        def slot_for(node):
            bw = last_of.get(int(node), -1)
            if bw < 0 or not mask[bw]:
                return M - 1
            s = slot_of.get(bw, -1)
            if s < 0:
                s = len(writers)
                slot_of[bw] = s
                writers.append(bw)
            return s

        slot_h = np.empty(BC, np.int32)
        slot_t = np.empty(BC, np.int32)
        for i in range(BC):
            slot_h[i] = slot_for(head[lo + i])
            slot_t[i] = slot_for(tailv[lo + i])
        m = len(writers)
        assert m <= M - 1, f"core {c}: {m} comp slots exceed capacity {M - 1}"

        neigh = np.zeros((M, K), np.int32)      # pad -> node 0 (w = 0)
        wts = np.zeros((M, K), np.float32)
        deg = np.zeros(M, np.int32)
        valid = np.zeros(M, np.float32)
        wl = np.array(writers, dtype=np.int64)
        if m:
            ls = np.asarray(local_idx_map)[head[wl]]
            neigh[:m] = np.asarray(sim_neighbors)[ls]
            wts[:m] = np.asarray(sim_weights)[ls]
            deg[:m] = np.asarray(degree_table)[ls, rel[wl] - 2]
            valid[:m] = 1.0

        # gather column q covers slots 2q (partitions 0..63, k=p) and 2q+1
        # (partitions 64..127, k=p-64)
        idx_pair = np.ascontiguousarray(
            neigh.reshape(NQ, 2 * K).T)                        # [128, NQ]
        w2 = np.zeros((128, 2 * NQ), np.float32)
        w2[0:K, 0::2] = wts[0::2].T
        w2[K:128, 1::2] = wts[1::2].T

        cores.append(dict(
            idx_pair=idx_pair.astype(np.int32),
            w2=w2,
            deg_sb=np.ascontiguousarray(deg.reshape(NT, 128).T).astype(np.int32),
            valid_sb=np.ascontiguousarray(valid.reshape(NT, 128).T),
            headi=np.ascontiguousarray(
                head[lo:lo + BC].reshape(NJ, 128).T).astype(np.int32),
            taili=np.ascontiguousarray(
                tailv[lo:lo + BC].reshape(NJ, 128).T).astype(np.int32),
            reli=np.ascontiguousarray(
                rel[lo:lo + BC].reshape(NJ, 128).T).astype(np.int32),
            sloth=np.ascontiguousarray(slot_h.reshape(NJ, 128).T),
            slott=np.ascontiguousarray(slot_t.reshape(NJ, 128).T),
            n_slots=m,
        ))
    return cores


# ----------------------------------------------------------------------------
# device program
# ----------------------------------------------------------------------------
def _build_nc():
    import concourse.bass as bass
    import concourse.bacc as bacc
    import concourse.mybir as mybir
    import concourse.tile as tile
    from concourse.tile import add_dep_helper
    from concourse.masks import make_identity

    f32 = mybir.dt.float32
    i32 = mybir.dt.int32
    Alu = mybir.AluOpType
    Act = mybir.ActivationFunctionType

    nc = bacc.Bacc("TRN2", target_bir_lowering=False, debug=False,
                   num_devices=NCORES)

    node_emb = nc.dram_tensor("node_emb", [N, D], f32, kind="ExternalInput")
    rel_emb = nc.dram_tensor("rel_emb", [RELS, D], f32, kind="ExternalInput")
    idx_pair_t = nc.dram_tensor("idx_pair", [128, NQ], i32, kind="ExternalInput")
    w2_t = nc.dram_tensor("w2", [128, 2 * NQ], f32, kind="ExternalInput")
    deg_t = nc.dram_tensor("deg_sb", [128, NT], i32, kind="ExternalInput")
    valid_t = nc.dram_tensor("valid_sb", [128, NT], f32, kind="ExternalInput")
    headi_t = nc.dram_tensor("headi", [128, NJ], i32, kind="ExternalInput")
    taili_t = nc.dram_tensor("taili", [128, NJ], i32, kind="ExternalInput")
    reli_t = nc.dram_tensor("reli", [128, NJ], i32, kind="ExternalInput")
    sloth_t = nc.dram_tensor("sloth", [128, NJ], i32, kind="ExternalInput")
    slott_t = nc.dram_tensor("slott", [128, NJ], i32, kind="ExternalInput")
    scratch = nc.dram_tensor("scratch", [M, SC_W], f32, kind="Internal")
    score_t = nc.dram_tensor("score", [128, NJ], f32, kind="ExternalOutput")

    with tile.TileContext(nc) as tc:
        with tc.tile_pool(name="const", bufs=1) as constp, \
             tc.tile_pool(name="gath", bufs=8) as gathp, \
             tc.tile_pool(name="psum", bufs=2, space="PSUM") as psump, \
             tc.tile_pool(name="work", bufs=3) as workp, \
             tc.tile_pool(name="scoreg", bufs=1) as scorep:

            idx_sb = constp.tile([128, NQ], i32)
            nc.sync.dma_start(out=idx_sb[:], in_=idx_pair_t.ap())
            w2_sb = constp.tile([128, 2 * NQ], f32)
            nc.sync.dma_start(out=w2_sb[:], in_=w2_t.ap())
            deg_sb = constp.tile([128, NT], i32)
            nc.sync.dma_start(out=deg_sb[:], in_=deg_t.ap())
            valid_sb = constp.tile([128, NT], f32)
            nc.sync.dma_start(out=valid_sb[:], in_=valid_t.ap())
            headi_sb = constp.tile([128, NJ], i32)
            nc.sync.dma_start(out=headi_sb[:], in_=headi_t.ap())
            taili_sb = constp.tile([128, NJ], i32)
            nc.sync.dma_start(out=taili_sb[:], in_=taili_t.ap())
            reli_sb = constp.tile([128, NJ], i32)
            nc.sync.dma_start(out=reli_sb[:], in_=reli_t.ap())
            sloth_sb = constp.tile([128, NJ], i32)
            nc.sync.dma_start(out=sloth_sb[:], in_=sloth_t.ap())
            slott_sb = constp.tile([128, NJ], i32)
            nc.sync.dma_start(out=slott_sb[:], in_=slott_t.ap())

            # a = valid * (LAM * exp(-LAM * deg) + 0.2); g = 1 - a
            degf = constp.tile([128, NT], f32)
            nc.vector.tensor_copy(out=degf[:], in_=deg_sb[:])
            ev = constp.tile([128, NT], f32)
            nc.scalar.activation(out=ev[:], in_=degf[:], func=Act.Exp,
                                 scale=-LAM)
            cfull = constp.tile([128, NT], f32)
            nc.scalar.activation(out=cfull[:], in_=ev[:], func=Act.Copy,
                                 bias=0.2, scale=LAM)
            a_sb = constp.tile([128, NT], f32)
            nc.vector.tensor_tensor(out=a_sb[:], in0=cfull[:], in1=valid_sb[:],
                                    op=Alu.mult)
            g_sb = constp.tile([128, NT], f32)
            nc.scalar.activation(out=g_sb[:], in_=a_sb[:], func=Act.Copy,
                                 bias=1.0, scale=-1.0)

            ident = constp.tile([128, 128], f32)
            make_identity(nc, ident[:])

            # ---- comp phase ----
            # matmul: out[d, j] = sum_p gt[p, d] * w2[p, 2q+j]  (dv transposed)
            scratch_writes = []
            for t in range(NT):
                pt = psump.tile([128, 128], f32, tag="psum")
                for q0 in range(t * 64, (t + 1) * 64):
                    gt_ = gathp.tile([128, D], f32, tag="g")
                    nc.gpsimd.indirect_dma_start(
                        out=gt_[:], out_offset=None, in_=node_emb.ap(),
                        in_offset=bass.IndirectOffsetOnAxis(
                            ap=idx_sb[:, q0:q0 + 1], axis=0),
                    )
                    col = 2 * (q0 - t * 64)
                    nc.tensor.matmul(out=pt[:, col:col + 2],
                                     lhsT=gt_[:],
                                     rhs=w2_sb[:, 2 * q0:2 * q0 + 2],
                                     start=True, stop=True)
                dvt = workp.tile([128, 128], f32, tag="dvt")
                nc.vector.tensor_copy(out=dvt[:], in_=pt[:])
                pt2 = psump.tile([128, 128], f32, tag="psum2")
                nc.tensor.transpose(out=pt2[:], in_=dvt[:], identity=ident[:])
                sc = workp.tile([128, SC_W], f32, tag="sc")
                nc.vector.tensor_copy(out=sc[:, 0:D], in_=pt2[:])
                nc.vector.tensor_copy(out=sc[:, D:D + 1], in_=a_sb[:, t:t + 1])
                nc.vector.tensor_copy(out=sc[:, D + 1:D + 2],
                                      in_=g_sb[:, t:t + 1])
                nc.vector.memset(sc[:, D + 2:SC_W], 0.0)
                wi = nc.sync.dma_start(
                    out=scratch.ap()[t * 128:(t + 1) * 128, :], in_=sc[:])
                scratch_writes.append(wi)

            # ---- score phase ----
            score_sb = constp.tile([128, NJ], f32)
            for j in range(NJ):
                gh = scorep.tile([128, SC_W], f32, tag="gh")
                gih = nc.gpsimd.indirect_dma_start(
                    out=gh[:], out_offset=None, in_=scratch.ap(),
                    in_offset=bass.IndirectOffsetOnAxis(
                        ap=sloth_sb[:, j:j + 1], axis=0))
                gt2 = scorep.tile([128, SC_W], f32, tag="gt2")
                git = nc.gpsimd.indirect_dma_start(
                    out=gt2[:], out_offset=None, in_=scratch.ap(),
                    in_offset=bass.IndirectOffsetOnAxis(
                        ap=slott_sb[:, j:j + 1], axis=0))
                for gi in (gih, git):
                    for wi in scratch_writes:
                        add_dep_helper(gi.ins, wi.ins,
                                       reason="scratch RAW")
                oh = scorep.tile([128, D], f32, tag="oh")
                nc.gpsimd.indirect_dma_start(
                    out=oh[:], out_offset=None, in_=node_emb.ap(),
                    in_offset=bass.IndirectOffsetOnAxis(
                        ap=headi_sb[:, j:j + 1], axis=0))
                ot = scorep.tile([128, D], f32, tag="ot")
                nc.gpsimd.indirect_dma_start(
                    out=ot[:], out_offset=None, in_=node_emb.ap(),
                    in_offset=bass.IndirectOffsetOnAxis(
                        ap=taili_sb[:, j:j + 1], axis=0))
                rl = scorep.tile([128, D], f32, tag="rl")
                nc.gpsimd.indirect_dma_start(
                    out=rl[:], out_offset=None, in_=rel_emb.ap(),
                    in_offset=bass.IndirectOffsetOnAxis(
                        ap=reli_sb[:, j:j + 1], axis=0))

                t1 = workp.tile([128, D], f32, tag="t1")
                nc.vector.tensor_scalar(out=t1[:], in0=gh[:, 0:D],
                                        scalar1=gh[:, D:D + 1], scalar2=None,
                                        op0=Alu.mult)
                t2 = workp.tile([128, D], f32, tag="t2")
                nc.vector.tensor_scalar(out=t2[:], in0=oh[:],
                                        scalar1=gh[:, D + 1:D + 2],
                                        scalar2=None, op0=Alu.mult)
                hv = workp.tile([128, D], f32, tag="hv")
                nc.vector.tensor_tensor(out=hv[:], in0=t1[:], in1=t2[:],
                                        op=Alu.add)
                t3 = workp.tile([128, D], f32, tag="t3")
                nc.vector.tensor_scalar(out=t3[:], in0=gt2[:, 0:D],
                                        scalar1=gt2[:, D:D + 1], scalar2=None,
                                        op0=Alu.mult)
                t4 = workp.tile([128, D], f32, tag="t4")
                nc.vector.tensor_scalar(out=t4[:], in0=ot[:],
                                        scalar1=gt2[:, D + 1:D + 2],
                                        scalar2=None, op0=Alu.mult)
                tv = workp.tile([128, D], f32, tag="tv")
                nc.vector.tensor_tensor(out=tv[:], in0=t3[:], in1=t4[:],
                                        op=Alu.add)
                p1 = workp.tile([128, D], f32, tag="p1")
                nc.vector.tensor_tensor(out=p1[:], in0=hv[:], in1=tv[:],
                                        op=Alu.mult)
                p2 = workp.tile([128, D], f32, tag="p2")
                nc.vector.tensor_tensor(out=p2[:], in0=p1[:], in1=rl[:],
                                        op=Alu.mult)
                nc.vector.reduce_sum(out=score_sb[:, j:j + 1], in_=p2[:],
                                     axis=mybir.AxisListType.X)
            nc.sync.dma_start(out=score_t.ap(), in_=score_sb[:])

    nc.compile()
    return nc


def _get_nc():
    if "nc" not in _CACHE:
        _CACHE["nc"] = _build_nc()
    return _CACHE["nc"]


# ----------------------------------------------------------------------------
# entry point
# ----------------------------------------------------------------------------
def kernel(head_index, rel_type, tail_index, node_emb, rel_emb,
           local_idx_map, sim_neighbors, sim_weights, degree_table):
    from concourse.bass_utils import run_bass_kernel_spmd

    head = np.asarray(head_index).astype(np.int64)
    rel = np.asarray(rel_type).astype(np.int64)
    tailv = np.asarray(tail_index).astype(np.int64)
    node_emb = np.ascontiguousarray(np.asarray(node_emb, dtype=np.float32))
    rel_emb = np.ascontiguousarray(np.asarray(rel_emb, dtype=np.float32))

    cores = _prep_cores(head, rel, tailv, local_idx_map, sim_neighbors,
                        sim_weights, degree_table)

    nc = _get_nc()
    in_maps = []
    for c in range(NCORES):
        cc = cores[c]
        in_maps.append({
            "node_emb": node_emb,
            "rel_emb": rel_emb,
            "idx_pair": cc["idx_pair"],
            "w2": cc["w2"],
            "deg_sb": cc["deg_sb"],
            "valid_sb": cc["valid_sb"],
            "headi": cc["headi"],
            "taili": cc["taili"],
            "reli": cc["reli"],
            "sloth": cc["sloth"],
            "slott": cc["slott"],
        })

    _CACHE["last_in_maps"] = in_maps
    res = run_bass_kernel_spmd(nc, in_maps, core_ids=list(range(NCORES)))
    _CACHE["last_result"] = res

    out = np.empty(B, np.float32)
    for c in range(NCORES):
        out[c * BC:(c + 1) * BC] = res.results[c]["score"].T.reshape(-1)
    return out


# revision 6
# speedup vs baseline: 1.4422x; 1.4422x over previous
"""Trainium2 Bass kernel for nn_DistMultMod, v3 — dma_gather based.

Same decomposition as v1 (see kernel.py docstring): per-core comp slots
(masked last-writer nodes, ~200/core) with dv = sum_k w*node_emb[neigh],
then per-triplet blend h = a*dv + g*old and DistMult product-reduce.

v3 replaces the 232 per-op-1us indirect DMAs with a few large dma_gather
ops (SWDGE cost ~= 1us + 0.34ns/row regardless of row count):
  - comp neighbor rows: node_emb split into 16 blocks of 32768 rows (int16
    index space); comp rows bucketed by (parity tile T = slot%2, block b),
    one dma_gather per bucket at fixed 512-row capacity (pad rows = idx 0,
    w = 0).  Positions i -> (partition i%128, group i//128); groups are
    reduced by PE matmuls lhsT=W[128,128] (host-placed scaled one-hots,
    W[p, slot//2] = w) accumulating into psum[T] = dv rows.
  - scratch rows [256, 192] = [dv | a | g | 0...]; per-triplet slot rows and
    rel rows fetched with one dma_gather each.
  - old head/tail rows: 16 classic indirect gathers (int32 index space).
"""
import os
import numpy as np

B = 8192
NCORES = 8
BC = B // NCORES        # 1024 triplets per core
D = 128
K = 64
N = 500000
RELS = 16
ND = 20000
NBLK = 16               # node_emb row blocks of 32768 (int16 index space)
BLK = 32768
NJ = BC // 128          # score tiles = 8
SC_W = 192              # scratch row: 128 dv + a + g + pad (768B, %256==0)
LAM = 0.7
SP = True      # single_packet for dma_gather (HW-validated setting)
F32R = False   # fp32 matmuls (exact)

# defaults sized for uniform rel_type (masked fraction 3/16):
#   slots/core ~195 +- 13, rows per (T,b) bucket ~400 +- 20
M_DEF = 256             # comp slots (2 parity tiles x 128); last is dummy
GCAP_DEF = 4            # 128-row groups per bucket (512 rows capacity)

_CACHE = {}


def _prep_cores(head, rel, tailv, local_idx_map, sim_neighbors, sim_weights,
                degree_table, M, GCAP):
    NT = M // 128           # parity tiles: slot s -> (tile s%NT, partition s//NT)
    CAP = GCAP * 128        # rows per bucket
    mask = (rel >= 2) & (rel <= 4)
    local_idx_map = np.asarray(local_idx_map)
    sim_neighbors = np.asarray(sim_neighbors)
    sim_weights = np.asarray(sim_weights)
    degree_table = np.asarray(degree_table)

    last_of = {}
    for b in range(B):
        last_of[int(head[b])] = b

    cores = []
    for c in range(NCORES):
        lo = c * BC
        slot_of = {}
        writers = []

        def slot_for(node):
            bw = last_of.get(int(node), -1)
            if bw < 0 or not mask[bw]:
                return M - 1
            s = slot_of.get(bw, -1)
            if s < 0:
                s = len(writers)
                slot_of[bw] = s
                writers.append(bw)
            return s

        slot_h = np.empty(BC, np.int16)
        slot_t = np.empty(BC, np.int16)
        for i in range(BC):
            slot_h[i] = slot_for(head[lo + i])
            slot_t[i] = slot_for(tailv[lo + i])
        m = len(writers)
        if m > M - 1:
            return "slots"  # caller rebuilds with bigger M

        deg = np.zeros(M, np.int32)
        valid = np.zeros(M, np.float32)
        wl = np.array(writers, dtype=np.int64)
        if m:
            ls = local_idx_map[head[wl]]
            neigh_rows = sim_neighbors[ls].astype(np.int64)   # [m, K]
            w_rows = sim_weights[ls].astype(np.float32)       # [m, K]
            deg[:m] = degree_table[ls, rel[wl] - 2]
            valid[:m] = 1.0
        else:
            neigh_rows = np.zeros((0, K), np.int64)
            w_rows = np.zeros((0, K), np.float32)

        # ---- bucket comp rows by (T = slot%2, block) ----
        # per row: node r, weight w, psum partition col = slot//2
        srows = np.repeat(np.arange(m), K)                    # slot per row
        rnodes = neigh_rows.reshape(-1)
        wvals = w_rows.reshape(-1)
        Tpar = (srows % NT).astype(np.int64)
        blk = rnodes >> 15
        pcol = (srows // NT).astype(np.int64)
        off = (rnodes & (BLK - 1)).astype(np.int16)

        idx16 = np.zeros((NBLK * NT, CAP), np.int16)          # pad -> row 0
        wmat = np.zeros((NBLK * NT, GCAP, 128, 128), np.float32)
        order = np.lexsort((pcol, blk, Tpar))                 # T major, then b
        srt_T, srt_b = Tpar[order], blk[order]
        srt_off, srt_w, srt_p = off[order], wvals[order], pcol[order]
        for T in range(NT):
            for bk in range(NBLK):
                sel = np.flatnonzero((srt_T == T) & (srt_b == bk))
                nb = len(sel)
                if nb > CAP:
                    return "bucket"  # bucket overflow -> bigger GCAP
                op = T * NBLK + bk
                idx16[op, :nb] = srt_off[sel]
                pos = np.arange(nb)
                wmat[op, pos // 128, pos % 128, srt_p[sel]] = srt_w[sel]

        # wrapped int16 layout per op: [128, CAP//16], [p, s] = list[s*16+p]
        idx16_w = np.zeros((NBLK * NT, 128, CAP // 16), np.int16)
        for op in range(NBLK * NT):
            idx16_w[op] = np.tile(idx16[op].reshape(CAP // 16, 16).T, (8, 1))

        # slot/rel gather index lists (natural triplet order, wrapped)
        def wrap16(lst):
            n = len(lst)
            return np.tile(np.asarray(lst, np.int16).reshape(n // 16, 16).T,
                           (8, 1)).astype(np.int16)

        cores.append(dict(
            idx16=np.ascontiguousarray(
                idx16_w.transpose(1, 0, 2).reshape(128, NBLK * NT * (CAP // 16))),
            wmat=np.ascontiguousarray(
                wmat.reshape(NBLK * NT * GCAP, 128, 128)
                .transpose(1, 0, 2).reshape(128, NBLK * NT * GCAP * 128)),
            deg_sb=np.ascontiguousarray(
                deg.reshape(128, NT)).astype(np.int32),       # [p, T] = slot 2p+T
            valid_sb=np.ascontiguousarray(valid.reshape(128, NT)),
            sloth16=wrap16(slot_h),
            slott16=wrap16(slot_t),
            rel16=wrap16(rel[lo:lo + BC].astype(np.int16)),
            headi=np.ascontiguousarray(
                head[lo:lo + BC].reshape(NJ, 128).T).astype(np.int32),
            taili=np.ascontiguousarray(
                tailv[lo:lo + BC].reshape(NJ, 128).T).astype(np.int32),
            n_slots=m,
        ))
    return cores


def _build_nc(M, GCAP):
    import concourse.bass as bass
    import concourse.bacc as bacc
    import concourse.mybir as mybir
    import concourse.tile as tile
    from concourse.tile import add_dep_helper

    NT = M // 128
    CAP = GCAP * 128
    NOP = NBLK * NT
    f32 = mybir.dt.float32
    i32 = mybir.dt.int32
    i16 = mybir.dt.int16
    Alu = mybir.AluOpType
    Act = mybir.ActivationFunctionType

    nc = bacc.Bacc("TRN2", target_bir_lowering=False, debug=False,
                   num_devices=NCORES)

    node_emb = nc.dram_tensor("node_emb", [N, D], f32, kind="ExternalInput")
    rel_emb = nc.dram_tensor("rel_emb", [RELS, D], f32, kind="ExternalInput")
    idx16_t = nc.dram_tensor("idx16", [128, NOP * (CAP // 16)], i16,
                             kind="ExternalInput")
    wmat_t = nc.dram_tensor("wmat", [128, NOP * GCAP * 128], f32,
                            kind="ExternalInput")
    deg_t = nc.dram_tensor("deg_sb", [128, NT], i32, kind="ExternalInput")
    valid_t = nc.dram_tensor("valid_sb", [128, NT], f32, kind="ExternalInput")
    sloth_t = nc.dram_tensor("sloth16", [128, BC // 16], i16, kind="ExternalInput")
    slott_t = nc.dram_tensor("slott16", [128, BC // 16], i16, kind="ExternalInput")
    rel16_t = nc.dram_tensor("rel16", [128, BC // 16], i16, kind="ExternalInput")
    headi_t = nc.dram_tensor("headi", [128, NJ], i32, kind="ExternalInput")
    taili_t = nc.dram_tensor("taili", [128, NJ], i32, kind="ExternalInput")
    scratch = nc.dram_tensor("scratch", [M, SC_W], f32, kind="Internal")
    score_t = nc.dram_tensor("score", [128, NJ], f32, kind="ExternalOutput")

    with tile.TileContext(nc) as tc:
        with tc.tile_pool(name="const", bufs=1) as constp, \
             tc.tile_pool(name="gath", bufs=4) as gathp, \
             tc.tile_pool(name="wld", bufs=4) as wldp, \
             tc.tile_pool(name="psum", bufs=1, space="PSUM") as psump, \
             tc.tile_pool(name="work", bufs=3) as workp, \
             tc.tile_pool(name="scoreg", bufs=1) as scorep:

            idx_sb = constp.tile([128, NOP * (CAP // 16)], i16)
            nc.sync.dma_start(out=idx_sb[:], in_=idx16_t.ap())
            deg_sb = constp.tile([128, NT], i32)
            nc.sync.dma_start(out=deg_sb[:], in_=deg_t.ap())
            valid_sb = constp.tile([128, NT], f32)
            nc.sync.dma_start(out=valid_sb[:], in_=valid_t.ap())
            sloth_sb = constp.tile([128, BC // 16], i16)
            nc.sync.dma_start(out=sloth_sb[:], in_=sloth_t.ap())
            slott_sb = constp.tile([128, BC // 16], i16)
            nc.sync.dma_start(out=slott_sb[:], in_=slott_t.ap())
            rel16_sb = constp.tile([128, BC // 16], i16)
            nc.sync.dma_start(out=rel16_sb[:], in_=rel16_t.ap())
            headi_sb = constp.tile([128, NJ], i32)
            nc.sync.dma_start(out=headi_sb[:], in_=headi_t.ap())
            taili_sb = constp.tile([128, NJ], i32)
            nc.sync.dma_start(out=taili_sb[:], in_=taili_t.ap())

            # a = valid * (LAM*exp(-LAM*deg) + 0.2); g = 1 - a
            degf = constp.tile([128, NT], f32)
            nc.vector.tensor_copy(out=degf[:], in_=deg_sb[:])
            ev = constp.tile([128, NT], f32)
            nc.scalar.activation(out=ev[:], in_=degf[:], func=Act.Exp,
                                 scale=-LAM)
            cfull = constp.tile([128, NT], f32)
            nc.scalar.activation(out=cfull[:], in_=ev[:], func=Act.Copy,
                                 bias=0.2, scale=LAM)
            a_sb = constp.tile([128, NT], f32)
            nc.vector.tensor_tensor(out=a_sb[:], in0=cfull[:], in1=valid_sb[:],
                                    op=Alu.mult)
            g_sb = constp.tile([128, NT], f32)
            nc.scalar.activation(out=g_sb[:], in_=a_sb[:], func=Act.Copy,
                                 bias=1.0, scale=-1.0)

            # ---- comp phase: NOP dma_gathers + GCAP matmuls each ----
            psts = []
            for T in range(NT):
                ps_tile = psump.tile([128, 128], f32, tag=f"ps{T}",
                                     name=f"ps{T}")
                psts.append(ps_tile)
            scratch_writes = []
            for T in range(NT):
                for bk in range(NBLK):
                    op = T * NBLK + bk
                    gt_ = gathp.tile([128, GCAP * D], f32, tag="g")
                    nc.gpsimd.dma_gather(
                        out_ap=gt_[:].rearrange("p (b d) -> p b d", d=D),
                        in_ap=node_emb.ap()[bk * BLK:min((bk + 1) * BLK, N), :],
                        idxs_ap=idx_sb[:, op * (CAP // 16):(op + 1) * (CAP // 16)],
                        num_idxs=CAP, num_idxs_reg=CAP, elem_size=D,
                        single_packet=SP,
                    )
                    wt_ = wldp.tile([128, GCAP * 128], f32, tag="w")
                    nc.sync.dma_start(
                        out=wt_[:],
                        in_=wmat_t.ap()[:, op * GCAP * 128:(op + 1) * GCAP * 128])
                    for g in range(GCAP):
                        lh = wt_[:, g * 128:(g + 1) * 128]
                        rh = gt_[:, g * D:(g + 1) * D]
                        if F32R:
                            lh = lh.bitcast(mybir.dt.float32r)
                            rh = rh.bitcast(mybir.dt.float32r)
                        nc.tensor.matmul(
                            out=psts[T][:],
                            lhsT=lh,
                            rhs=rh,
                            start=(bk == 0 and g == 0),
                            stop=(bk == NBLK - 1 and g == GCAP - 1))
                sc = workp.tile([128, SC_W], f32, tag="sc")
                nc.vector.tensor_copy(out=sc[:, 0:D], in_=psts[T][:])
                nc.vector.tensor_copy(out=sc[:, D:D + 1], in_=a_sb[:, T:T + 1])
                nc.vector.tensor_copy(out=sc[:, D + 1:D + 2],
                                      in_=g_sb[:, T:T + 1])
                nc.vector.memset(sc[:, D + 2:SC_W], 0.0)
                wi = nc.sync.dma_start(
                    out=scratch.ap().rearrange("(p two) w -> two p w", two=NT)[T],
                    in_=sc[:])
                scratch_writes.append(wi)

            # ---- score phase ----
            gh = scorep.tile([128, NJ * SC_W], f32, tag="gh")
            gih = nc.gpsimd.dma_gather(
                out_ap=gh[:].rearrange("p (b w) -> p b w", w=SC_W),
                in_ap=scratch.ap(), idxs_ap=sloth_sb[:],
                num_idxs=BC, num_idxs_reg=BC, elem_size=SC_W,
                single_packet=SP)
            gt2 = scorep.tile([128, NJ * SC_W], f32, tag="gt2")
            git = nc.gpsimd.dma_gather(
                out_ap=gt2[:].rearrange("p (b w) -> p b w", w=SC_W),
                in_ap=scratch.ap(), idxs_ap=slott_sb[:],
                num_idxs=BC, num_idxs_reg=BC, elem_size=SC_W,
                single_packet=SP)
            for gi in (gih, git):
                for wi in scratch_writes:
                    add_dep_helper(gi.ins, wi.ins, reason="scratch RAW")
            rl = scorep.tile([128, NJ * D], f32, tag="rl")
            nc.gpsimd.dma_gather(
                out_ap=rl[:].rearrange("p (b d) -> p b d", d=D),
                in_ap=rel_emb.ap(), idxs_ap=rel16_sb[:],
                num_idxs=BC, num_idxs_reg=BC, elem_size=D,
                single_packet=SP)

            score_sb = constp.tile([128, NJ], f32)
            for j in range(NJ):
                oh = gathp.tile([128, D], f32, tag="oh")
                nc.gpsimd.indirect_dma_start(
                    out=oh[:], out_offset=None, in_=node_emb.ap(),
                    in_offset=bass.IndirectOffsetOnAxis(
                        ap=headi_sb[:, j:j + 1], axis=0))
                ot = gathp.tile([128, D], f32, tag="ot")
                nc.gpsimd.indirect_dma_start(
                    out=ot[:], out_offset=None, in_=node_emb.ap(),
                    in_offset=bass.IndirectOffsetOnAxis(
                        ap=taili_sb[:, j:j + 1], axis=0))

                t1 = workp.tile([128, D], f32, tag="t1")
                nc.vector.tensor_scalar(
                    out=t1[:], in0=gh[:, j * SC_W:j * SC_W + D],
                    scalar1=gh[:, j * SC_W + D:j * SC_W + D + 1], scalar2=None,
                    op0=Alu.mult)
                t2 = workp.tile([128, D], f32, tag="t2")
                nc.vector.tensor_scalar(
                    out=t2[:], in0=oh[:],
                    scalar1=gh[:, j * SC_W + D + 1:j * SC_W + D + 2],
                    scalar2=None, op0=Alu.mult)
                hv = workp.tile([128, D], f32, tag="hv")
                nc.vector.tensor_tensor(out=hv[:], in0=t1[:], in1=t2[:],
                                        op=Alu.add)
                t3 = workp.tile([128, D], f32, tag="t3")
                nc.vector.tensor_scalar(
                    out=t3[:], in0=gt2[:, j * SC_W:j * SC_W + D],
                    scalar1=gt2[:, j * SC_W + D:j * SC_W + D + 1], scalar2=None,
                    op0=Alu.mult)
                t4 = workp.tile([128, D], f32, tag="t4")
                nc.vector.tensor_scalar(
                    out=t4[:], in0=ot[:],
                    scalar1=gt2[:, j * SC_W + D + 1:j * SC_W + D + 2],
                    scalar2=None, op0=Alu.mult)
                tv = workp.tile([128, D], f32, tag="tv")
                nc.vector.tensor_tensor(out=tv[:], in0=t3[:], in1=t4[:],
                                        op=Alu.add)
                p1 = workp.tile([128, D], f32, tag="p1")
                nc.vector.tensor_tensor(out=p1[:], in0=hv[:], in1=tv[:],
                                        op=Alu.mult)
                p2 = workp.tile([128, D], f32, tag="p2")
                nc.vector.tensor_tensor(out=p2[:], in0=p1[:],
                                        in1=rl[:, j * D:(j + 1) * D],
                                        op=Alu.mult)
                nc.vector.reduce_sum(out=score_sb[:, j:j + 1], in_=p2[:],
                                     axis=mybir.AxisListType.X)
            nc.sync.dma_start(out=score_t.ap(), in_=score_sb[:])

    nc.compile()
    return nc


def _get_nc(M, GCAP):
    key = (M, GCAP)
    if key not in _CACHE:
        _CACHE[key] = _build_nc(M, GCAP)
    return _CACHE[key]


def kernel(head_index, rel_type, tail_index, node_emb, rel_emb,
           local_idx_map, sim_neighbors, sim_weights, degree_table):
    from concourse.bass_utils import run_bass_kernel_spmd

    head = np.asarray(head_index).astype(np.int64)
    rel = np.asarray(rel_type).astype(np.int64)
    tailv = np.asarray(tail_index).astype(np.int64)
    node_emb = np.ascontiguousarray(np.asarray(node_emb, dtype=np.float32))
    rel_emb = np.ascontiguousarray(np.asarray(rel_emb, dtype=np.float32))

    M, GCAP = M_DEF, GCAP_DEF
    while True:
        cores = _prep_cores(head, rel, tailv, local_idx_map, sim_neighbors,
                            sim_weights, degree_table, M, GCAP)
        if isinstance(cores, list):
            break
        if cores == "slots":   # rare: unusual input distribution
            M *= 2
        else:
            GCAP += 2

    nc = _get_nc(M, GCAP)
    in_maps = []
    for c in range(NCORES):
        cc = cores[c]
        in_maps.append({
            "node_emb": node_emb, "rel_emb": rel_emb,
            "idx16": cc["idx16"], "wmat": cc["wmat"],
            "deg_sb": cc["deg_sb"], "valid_sb": cc["valid_sb"],
            "sloth16": cc["sloth16"], "slott16": cc["slott16"],
            "rel16": cc["rel16"], "headi": cc["headi"], "taili": cc["taili"],
        })

    _CACHE["last_in_maps"] = in_maps
    res = run_bass_kernel_spmd(nc, in_maps, core_ids=list(range(NCORES)))
    _CACHE["last_result"] = res
    _CACHE["last_nc"] = nc

    out = np.empty(B, np.float32)
    for c in range(NCORES):
        out[c * BC:(c + 1) * BC] = res.results[c]["score"].T.reshape(-1)
    return out
